# revision 5
# baseline (speedup 1.0000x reference)
"""Trainium2 Bass kernel for nn_Decoder: measure-LSTM -> beat-LSTM -> linear.

Wall-clock-optimized single-core design.  The axon host<->device link runs at
~58 MB/s up / ~18 MB/s down, so total bytes moved dominates the end-to-end
time; device FLOPs are noise by comparison.  Running on ONE core means the
weights ship once instead of 8x (the data-parallel shard_map path concatenates
per-core inputs, so replicated weights are physically transferred per core),
while the unique activations (latent/inputs/y) are the same bytes either way.

Kernel layout: gates live on partitions, batch (256) is the moving dim.
  gates[g, b] = sum_k W^T-chunk[k, g-tile] . hT[k, b]   (f32r recurrent)
so the hidden state stays in [hidden, batch] layout across the whole scan and
no per-step transposes are needed.  The jnp.repeat structure is exploited by
precomputing lm[m] = bWih[:, :H] @ tanh(h_m) + bb once per measure during the
measure scan, then reusing it for that measure's 16 beat steps.  The output
GEMM (tanh(h) @ linW.T + linb) is fused into the beat scan per step.

Precision: recurrent beat path f32r (matches reference closely over the
512-step scan); all input-side projections bf16; output fetched as bf16.
"""

import sys

for _p in ("/opt/trn_rl_repo",):
    if _p not in sys.path:
        sys.path.insert(0, _p)

import numpy as np
import ml_dtypes

B, M, S = 256, 32, 16
IN, H, O = 512, 1024, 128
G = 4 * H            # 4096
T = M * S            # 512
KH = H // 128        # 8 hidden chunks
NGT = G // 128       # 32 gate tiles (quad q of hidden tile n = tiles 4n..4n+3)


def _gate_perm():
    """New gate index g_new = n*512 + q*128 + r  ->  original row q*H + n*128 + r.

    Groups the gates so that gate-tile 4n+q holds gate-kind q (i,f,g,o) of
    hidden chunk n; the c/h elementwise update for chunk n then reads four
    adjacent gate tiles and writes h partitions that line up with hT[:, n, :].
    """
    idx = np.arange(G)
    n = idx >> 9
    q = (idx >> 7) & 3
    r = idx & 127
    return q * H + n * 128 + r


def _build_nc():
    import concourse.bass as bass
    import concourse.mybir as mybir
    import concourse.tile as tile
    from concourse import bacc
    from concourse.bass import ds

    f32 = mybir.dt.float32
    f32r = mybir.dt.float32r
    bf16 = mybir.dt.bfloat16
    ACTF = mybir.ActivationFunctionType
    PSUM = bass.MemorySpace.PSUM

    nc = bacc.Bacc("TRN2", target_bir_lowering=False)

    latentT_d = nc.dram_tensor("latentT", [128, 4, M, B], bf16, kind="ExternalInput")
    inputsT_d = nc.dram_tensor("inputsT", [128, M, S, B], bf16, kind="ExternalInput")
    mWihT_d = nc.dram_tensor("mWihT", [128, 4, G], bf16, kind="ExternalInput")
    mWhhT_d = nc.dram_tensor("mWhhT", [128, KH, G], bf16, kind="ExternalInput")
    mbT_d = nc.dram_tensor("mbT", [128, NGT], f32, kind="ExternalInput")
    bWih1T_d = nc.dram_tensor("bWih1T", [128, KH, G], bf16, kind="ExternalInput")
    bWih2T_d = nc.dram_tensor("bWih2T", [128, G], f32r, kind="ExternalInput")
    bbT_d = nc.dram_tensor("bbT", [128, NGT], f32, kind="ExternalInput")
    bWhhT_d = nc.dram_tensor("bWhhT", [128, KH, G], f32r, kind="ExternalInput")
    linWT_d = nc.dram_tensor("linWT", [128, KH, O], bf16, kind="ExternalInput")
    linb_d = nc.dram_tensor("linb", [O, 1], f32, kind="ExternalInput")

    yT_d = nc.dram_tensor("yT", [O, M, S, B], bf16, kind="ExternalOutput")
    lm_d = nc.dram_tensor("lm_scr", [M, 128, NGT, B], bf16, kind="Internal")

    with tile.TileContext(nc) as tc:

        # ================= Phase A: measure scan (+ lm precompute) ========
        with (
            tc.tile_pool(name="a_w", bufs=1) as wpool,
            tc.tile_pool(name="a_st", bufs=1) as spool,
            tc.tile_pool(name="a_gp", bufs=2, space=PSUM) as gppool,
            tc.tile_pool(name="a_lp", bufs=2, space=PSUM) as lmppool,
            tc.tile_pool(name="a_ew", bufs=2) as ewpool,
            tc.tile_pool(name="a_in", bufs=2) as inpool,
        ):
            mwhh = wpool.tile([128, KH, G], bf16)
            nc.sync.dma_start(out=mwhh[:], in_=mWhhT_d[:])
            mwih = wpool.tile([128, 4, G], bf16)
            nc.sync.dma_start(out=mwih[:], in_=mWihT_d[:])
            bwih1 = wpool.tile([128, KH, G], bf16)
            nc.sync.dma_start(out=bwih1[:], in_=bWih1T_d[:])
            mbT = wpool.tile([128, NGT], f32)
            nc.sync.dma_start(out=mbT[:], in_=mbT_d[:])
            bbT = wpool.tile([128, NGT], f32)
            nc.sync.dma_start(out=bbT[:], in_=bbT_d[:])

            hT_a = spool.tile([128, KH, B], bf16)
            hT_b = spool.tile([128, KH, B], bf16)
            c_sb = spool.tile([128, KH, B], f32)
            lat = spool.tile([128, KH, B], bf16)
            nc.vector.memset(hT_a[:], 0.0)
            nc.vector.memset(c_sb[:], 0.0)

            def a_step(m_ap, hT_r, hT_w):
                xin = inpool.tile([128, 4, B], bf16, tag="xin")
                nc.sync.dma_start(out=xin[:], in_=latentT_d[:, :, m_ap, :])
                for n in range(KH):
                    qp = gppool.tile([128, 4, B], f32, tag="qp")
                    for g4 in range(4):
                        gsl = slice((4 * n + g4) * 128, (4 * n + g4 + 1) * 128)
                        for k in range(KH):
                            nc.tensor.matmul(qp[:, g4, :], mwhh[:, k, gsl],
                                             hT_r[:, k, :],
                                             start=(k == 0), stop=False)
                        for kc in range(4):
                            nc.tensor.matmul(qp[:, g4, :], mwih[:, kc, gsl],
                                             xin[:, kc, :],
                                             start=False, stop=(kc == 3))
                    ga = ewpool.tile([128, 4, B], f32, tag="ga")
                    for g4, fn in enumerate((ACTF.Sigmoid, ACTF.Sigmoid,
                                             ACTF.Tanh, ACTF.Sigmoid)):
                        gt = 4 * n + g4
                        nc.scalar.activation(ga[:, g4, :], qp[:, g4, :], fn,
                                             bias=mbT[:, gt:gt + 1])
                    t1 = ewpool.tile([128, B], f32, tag="t1")
                    t2 = ewpool.tile([128, B], f32, tag="t2")
                    nc.vector.tensor_mul(t1[:], ga[:, 0, :], ga[:, 2, :])
                    nc.vector.tensor_mul(t2[:], ga[:, 1, :], c_sb[:, n, :])
                    nc.vector.tensor_add(c_sb[:, n, :], t1[:], t2[:])
                    nc.scalar.activation(t2[:], c_sb[:, n, :], ACTF.Tanh)
                    nc.vector.tensor_mul(hT_w[:, n, :], ga[:, 3, :], t2[:])
                    nc.scalar.activation(lat[:, n, :], hT_w[:, n, :], ACTF.Tanh)
                # lm[m] = bWih[:, :H] @ lat + bb, written once per measure
                for qn in range(KH):
                    lp = lmppool.tile([128, 4, B], f32, tag="lp")
                    for g4 in range(4):
                        gsl = slice((4 * qn + g4) * 128, (4 * qn + g4 + 1) * 128)
                        for k in range(KH):
                            nc.tensor.matmul(lp[:, g4, :], bwih1[:, k, gsl],
                                             lat[:, k, :],
                                             start=(k == 0), stop=(k == KH - 1))
                    lms = ewpool.tile([128, 4, B], bf16, tag="lms")
                    for g4 in range(4):
                        gt = 4 * qn + g4
                        nc.scalar.activation(lms[:, g4, :], lp[:, g4, :],
                                             ACTF.Identity, bias=bbT[:, gt:gt + 1])
                    nc.sync.dma_start(out=lm_d[m_ap, :, 4 * qn:4 * qn + 4, :],
                                      in_=lms[:])

            with tc.For_i(0, M, 2) as m0:
                a_step(ds(m0, 1), hT_a, hT_b)
                a_step(ds(m0 + 1, 1), hT_b, hT_a)

        # ================= Phase C: beat scan (output GEMM fused) =========
        with (
            tc.tile_pool(name="c_w", bufs=1) as wpool,
            tc.tile_pool(name="c_st", bufs=1) as spool,
            tc.tile_pool(name="c_gp", bufs=3, space=PSUM) as gppool,
            tc.tile_pool(name="c_yp", bufs=2, space=PSUM) as yppool,
            tc.tile_pool(name="c_ew", bufs=2) as ewpool,
            tc.tile_pool(name="c_in", bufs=2) as inpool,
            tc.tile_pool(name="c_lm", bufs=1) as lmpool,
        ):
            bwhh = wpool.tile([128, KH, G], f32r)
            nc.sync.dma_start(out=bwhh[:], in_=bWhhT_d[:])
            bwih2 = wpool.tile([128, G], f32r)
            nc.sync.dma_start(out=bwih2[:], in_=bWih2T_d[:])
            lw = wpool.tile([128, KH, O], bf16)
            nc.sync.dma_start(out=lw[:], in_=linWT_d[:])
            lb = wpool.tile([O, 1], f32)
            nc.sync.dma_start(out=lb[:], in_=linb_d[:])

            hT_a = spool.tile([128, KH, B], f32r)
            hT_b = spool.tile([128, KH, B], f32r)
            c_sb = spool.tile([128, KH, B], f32)
            th = spool.tile([128, KH, B], bf16)
            nc.vector.memset(c_sb[:], 0.0)
            # f32r tiles may not be memset directly; copy rounded zeros in
            nc.scalar.activation(hT_a[:], c_sb[:], ACTF.Copy)

            def c_step(m_ap, j_ap, lm_sb, hT_r, hT_w):
                xin = inpool.tile([128, B], bf16, tag="xin")
                nc.sync.dma_start(out=xin[:], in_=inputsT_d[:, m_ap, j_ap, :])
                xinr = inpool.tile([128, B], f32r, tag="xinr")
                nc.scalar.activation(xinr[:], xin[:], ACTF.Copy)
                for n in range(KH):
                    qp = gppool.tile([128, 4, B], f32, tag="qp")
                    for g4 in range(4):
                        gsl = slice((4 * n + g4) * 128, (4 * n + g4 + 1) * 128)
                        for k in range(KH):
                            nc.tensor.matmul(qp[:, g4, :], bwhh[:, k, gsl],
                                             hT_r[:, k, :],
                                             start=(k == 0), stop=False)
                        nc.tensor.matmul(qp[:, g4, :], bwih2[:, gsl], xinr[:],
                                         start=False, stop=True)
                    ga = ewpool.tile([128, 4, B], f32, tag="ga")
                    nc.vector.tensor_add(ga[:], qp[:], lm_sb[:, 4 * n:4 * n + 4, :])
                    nc.scalar.activation(ga[:, 0:2, :], ga[:, 0:2, :], ACTF.Sigmoid)
                    nc.scalar.activation(ga[:, 2, :], ga[:, 2, :], ACTF.Tanh)
                    nc.scalar.activation(ga[:, 3, :], ga[:, 3, :], ACTF.Sigmoid)
                    t1 = ewpool.tile([128, B], f32, tag="t1")
                    t2 = ewpool.tile([128, B], f32, tag="t2")
                    nc.vector.tensor_mul(t1[:], ga[:, 0, :], ga[:, 2, :])
                    nc.vector.tensor_mul(t2[:], ga[:, 1, :], c_sb[:, n, :])
                    nc.vector.tensor_add(c_sb[:, n, :], t1[:], t2[:])
                    nc.scalar.activation(t2[:], c_sb[:, n, :], ACTF.Tanh)
                    nc.vector.tensor_mul(hT_w[:, n, :], ga[:, 3, :], t2[:])
                    nc.scalar.activation(th[:, n, :], hT_w[:, n, :], ACTF.Tanh)
                yp = yppool.tile([O, B], f32, tag="yp")
                for n in range(KH):
                    nc.tensor.matmul(yp[:], lw[:, n, :], th[:, n, :],
                                     start=(n == 0), stop=(n == KH - 1))
                ysb = ewpool.tile([O, B], bf16, tag="ysb")
                nc.scalar.activation(ysb[:], yp[:], ACTF.Identity, bias=lb[:])
                nc.sync.dma_start(out=yT_d[:, m_ap, j_ap, :], in_=ysb[:])

            with tc.For_i(0, M) as m0:
                lm_sb = lmpool.tile([128, NGT, B], bf16, tag="lm")
                nc.sync.dma_start(out=lm_sb[:], in_=lm_d[ds(m0, 1), :, :, :])
                with tc.For_i(0, S, 2) as j0:
                    c_step(ds(m0, 1), ds(j0, 1), lm_sb, hT_a, hT_b)
                    c_step(ds(m0, 1), ds(j0 + 1, 1), lm_sb, hT_b, hT_a)

    nc.compile()
    return nc


def _prep_inputs(latent, inputs, mWih, mWhh, mb, bWih, bWhh, bb, linW, linb):
    bf = ml_dtypes.bfloat16
    src = _gate_perm()

    def wT(w, kchunks, dtype):
        return np.ascontiguousarray(
            w[src].T.reshape(kchunks, 128, G).transpose(1, 0, 2)).astype(dtype)

    in_map = {
        "latentT": np.ascontiguousarray(
            latent.transpose(2, 1, 0).reshape(4, 128, M, B)
            .transpose(1, 0, 2, 3)).astype(bf),
        "inputsT": np.ascontiguousarray(inputs.transpose(3, 1, 2, 0)).astype(bf),
        "mWihT": wT(mWih, 4, bf),
        "mWhhT": wT(mWhh, KH, bf),
        "mbT": np.ascontiguousarray(mb[src].reshape(NGT, 128).T).astype(np.float32),
        "bWih1T": wT(bWih[:, :H], KH, bf),
        "bWih2T": np.ascontiguousarray(bWih[src][:, H:].T).astype(np.float32),
        "bbT": np.ascontiguousarray(bb[src].reshape(NGT, 128).T).astype(np.float32),
        "bWhhT": wT(bWhh, KH, np.float32),
        "linWT": np.ascontiguousarray(
            linW.T.reshape(KH, 128, O).transpose(1, 0, 2)).astype(bf),
        "linb": np.ascontiguousarray(linb.reshape(O, 1)).astype(np.float32),
    }
    return in_map


def _run_fast(in_map):
    """Single-core run with the host->device upload overlapped with the Bass
    trace + neuronx-cc compile (the upload is the second-largest cost after
    compile; jax.device_put is async)."""
    import jax
    import jax.numpy as jnp
    from concourse import bass2jax
    import concourse.mybir as mybir

    dev = jax.devices()[0]
    names = list(in_map.keys())
    dev_arrs = dict(zip(names, jax.device_put([in_map[n] for n in names], dev)))

    nc = _build_nc()

    bass2jax.install_neuronx_cc_hook()
    assert nc.dbg_addr is None
    partition_name = (nc.partition_id_tensor.name
                      if nc.partition_id_tensor else None)

    in_names, out_names, out_avals = [], [], []
    for alloc in nc.m.functions[0].allocations:
        if not isinstance(alloc, mybir.MemoryLocationSet):
            continue
        name = alloc.memorylocations[0].name
        if alloc.kind == "ExternalInput":
            if name != partition_name:
                in_names.append(name)
        elif alloc.kind == "ExternalOutput":
            out_names.append(name)
            out_avals.append(jax.core.ShapedArray(
                tuple(alloc.tensor_shape), mybir.dt.np(alloc.dtype)))
    assert set(in_names) == set(names), (in_names, names)
    n_params = len(in_names)
    donate = tuple(range(n_params, n_params + len(out_names)))

    all_names = in_names + out_names + ([partition_name] if partition_name
                                        else [])

    def _body(*args):
        operands = list(args)
        if partition_name is not None:
            operands.append(bass2jax.partition_id_tensor())
        outs = bass2jax._bass_exec_p.bind(
            *operands,
            out_avals=tuple(out_avals),
            in_names=tuple(all_names),
            out_names=tuple(out_names),
            lowering_input_output_aliases=(),
            sim_require_finite=True,
            sim_require_nnan=True,
            nc=nc,
        )
        return tuple(outs)

    with jax.default_device(dev):
        zeros = [jnp.zeros(a.shape, a.dtype) for a in out_avals]
    fn = jax.jit(_body, donate_argnums=donate, keep_unused=True)
    outs = fn(*[dev_arrs[n] for n in in_names], *zeros)
    return {name: np.asarray(outs[i]) for i, name in enumerate(out_names)}


def _run_fallback(in_map):
    from concourse.bass_utils import run_bass_kernel_spmd

    nc = _build_nc()
    res = run_bass_kernel_spmd(nc, [dict(in_map)], core_ids=[0])
    return res.results[0]


def kernel(latent, inputs, mWih, mWhh, mb, bWih, bWhh, bb, linW, linb):
    in_map = _prep_inputs(latent, inputs, mWih, mWhh, mb, bWih, bWhh, bb,
                          linW, linb)
    try:
        outs = _run_fast(in_map)
    except Exception:
        import traceback
        traceback.print_exc()
        print("kernel: fast path failed, using run_bass_kernel_spmd fallback",
              flush=True)
        outs = _run_fallback(in_map)
    yT = outs["yT"]                                   # [O, M, S, B] bf16
    y = yT.reshape(O, T, B).transpose(2, 1, 0).astype(np.float32)
    return np.ascontiguousarray(y)
